# revision 16
# baseline (speedup 1.0000x reference)
"""Trainium2 Bass kernel for CompositionalFC (moe_routing).

Reference computation:
    z[n,b,o] = x[b,i] @ weight[n,i,o] + bias[n,o]
    out[b,o] = relu( sum_n comp_weight[b,n] * z[n,b,o] )

Strategy: data-parallel over batch across 8 NeuronCores (512 rows each,
weight/bias replicated). The matmuls run in fp8-e4m3 with
perf_mode=DoubleRow (two k-tiles per pass, ~1.8x bf16 PE throughput).
To keep fp8 error inside the tolerance the weights are mean-centered
before quantization:  W = 0.5 + V,  V in [-0.5, 0.5]  (torch.rand init),
so  x @ W = 0.5 * rowsum(x) + x @ V.  The fp8 matmuls only compute the
small zero-mean  x @ V  part.  The rank-1 mean term
t[b] = 0.5 * rowsum(x)[b] * rowsum(c)[b]  is applied at the very end by
the otherwise-idle Scalar engine:  out = relu(acc + t)  (per-partition
bias), so its inputs (a bf16 copy of x, reduced on the DVE mid-kernel)
never touch the startup critical path.  The bias term (c @ bias) seeds
the fp32 accumulators via a K=16 matmul.  Per expert the PSUM partial
z_n is combined into fp32 SBUF accumulators with a single fused DVE op
acc = z*c[:,n] + acc.
"""

import sys

for _p in ("/opt/trn_rl_repo",):
    if _p not in sys.path:
        sys.path.insert(0, _p)

from contextlib import ExitStack

import ml_dtypes
import numpy as np

import concourse.bass as bass
import concourse.mybir as mybir
import concourse.tile as tile
from concourse import bacc
from concourse.bass_utils import run_bass_kernel_spmd
from concourse.tile_rust import add_dep_helper

N_CORES = 8
BATCH, IN_DIM, OUT_DIM, N_EXP = 4096, 1024, 1024, 16
BS = BATCH // N_CORES          # 512 batch rows per core
P = 128                        # partitions
BT = BS // P                   # 4 batch tiles per core
KT = IN_DIM // P               # 8 contraction tiles per expert
NPAIR = KT // 2                # 4 DoubleRow k-tile pairs
FD = 512                       # matmul free dim / PSUM bank width (fp32)
NO = OUT_DIM // FD             # 2 output column tiles

F32 = mybir.dt.float32
BF16 = mybir.dt.bfloat16
FP8 = mybir.dt.float8e4
DR = mybir.MatmulPerfMode.DoubleRow


def _build_kernel():
    nc = bacc.Bacc(
        "TRN2",
        target_bir_lowering=False,
        debug=False,
        num_devices=N_CORES,
    )
    xT = nc.declare_dram_parameter("xT", [IN_DIM, BS], FP8, isOutput=False)
    x16 = nc.declare_dram_parameter("x16", [BS, IN_DIM], BF16, isOutput=False)
    w = nc.declare_dram_parameter("w", [N_EXP, IN_DIM, OUT_DIM], FP8, isOutput=False)
    c = nc.declare_dram_parameter("c", [BS, N_EXP], F32, isOutput=False)
    cT = nc.declare_dram_parameter("cT", [N_EXP, BS], BF16, isOutput=False)
    bias = nc.declare_dram_parameter("bias", [N_EXP, OUT_DIM], BF16, isOutput=False)
    out = nc.declare_dram_parameter("out", [BS, OUT_DIM], F32, isOutput=True)

    with ExitStack() as ctx:
        tc = ctx.enter_context(tile.TileContext(nc))
        const = ctx.enter_context(tc.tile_pool(name="const", bufs=1))
        accp = ctx.enter_context(tc.tile_pool(name="accp", bufs=1))
        wpool = ctx.enter_context(tc.tile_pool(name="wpool", bufs=3))
        psum = ctx.enter_context(tc.tile_pool(name="psum", bufs=4, space="PSUM"))

        # --- startup-critical SBUF state -------------------------------
        # cT/bias feed the bias-seed matmuls that warm the PE while xT and
        # w[0] stream in; x16 (only needed by the final relu pass) is
        # deferred into the expert loop to keep it off the startup path.
        # Junk-warmup inputs: memset tiles need no DMA, so the PE's HAM
        # clock-gate warmup starts right after the preamble barrier instead
        # of waiting for the first HBM transfer.
        jl_sb = const.tile([P, P], BF16, tag="jl_sb")
        nc.gpsimd.memset(jl_sb[:], 0.0)
        jr_sb = const.tile([P, FD], BF16, tag="jr_sb")
        nc.gpsimd.memset(jr_sb[:], 0.0)

        cT_sb = const.tile([N_EXP, BS], BF16, tag="cT_sb")
        nc.sync.dma_start(cT_sb[:], cT[:, :])
        bias_sb = const.tile([N_EXP, OUT_DIM], BF16, tag="bias_sb")
        nc.sync.dma_start(bias_sb[:], bias[:, :])
        xT_sb = const.tile([P, KT, BS], FP8, tag="xT_sb")
        nc.sync.dma_start(xT_sb[:], xT[:, :].rearrange("(kt p) b -> p kt b", p=P))
        c_sb = const.tile([P, BT, N_EXP], F32, tag="c_sb")
        x16_sb = const.tile([P, BT, IN_DIM], BF16, tag="x16_sb")

        acc = [
            accp.tile([P, NO, FD], F32, name=f"acc_{bt}", tag=f"acc_{bt}")
            for bt in range(BT)
        ]

        # rank-1 mean correction pieces (all off the critical path):
        # t = (0.5 * rowsum(c)) * rowsum(x16)
        s_sb = const.tile([P, BT], F32, tag="s_sb")
        t_sb = const.tile([P, BT], F32, tag="t_sb")

        # --- HAM warm-up -----------------------------------------------
        # ~4.3us of DMA-independent matmuls on the memset tiles keep the PE
        # active from the moment the preamble barrier drops, so the clock
        # gate is (closer to) 8/8 when the real fp8 stream starts.
        junk = psum.tile([P, FD], F32, name="junk", tag="zp")
        for _ in range(2):
            nc.tensor.matmul(
                junk[:],
                lhsT=jl_sb[:],
                rhs=jr_sb[:],
                start=True,
                stop=True,
            )

        # --- seed accumulators: acc = c @ bias -------------------------
        # The psum->sbuf copy runs on the otherwise-idle Scalar engine so
        # the pt tiles recycle immediately (no DVE backlog in front).
        for bt in range(BT):
            pt = psum.tile([P, NO, FD], F32, name="pt_init", tag="zp")
            for ot in range(NO):
                nc.tensor.matmul(
                    pt[:, ot],
                    lhsT=cT_sb[:, bt * P : (bt + 1) * P],
                    rhs=bias_sb[:, ot * FD : (ot + 1) * FD],
                    start=True,
                    stop=True,
                )
            nc.scalar.copy(acc[bt][:], pt[:])

        # --- main expert loop ------------------------------------------
        # Expert 1's prefetch is gated behind expert 0's last chunk so the
        # startup-critical ~1.5 MiB (xT + w[0]) gets the full HBM bandwidth.
        w0_last_dma = None
        for n in range(N_EXP):
            w_sb = wpool.tile([P, KT, OUT_DIM], FP8, name="w_sb", tag="w_sb")
            for kt_i in range(KT):
                dma = nc.sync.dma_start(w_sb[:, kt_i], w[n, :, :].rearrange("(kt p) o -> p kt o", p=P)[:, kt_i])
                if n == 0 and kt_i == KT - 1:
                    w0_last_dma = dma
                if n == 1 and w0_last_dma is not None:
                    add_dep_helper(
                        dma.ins, w0_last_dma.ins, sync=True,
                        reason="gate w[1] prefetch behind startup-critical w[0]",
                    )
            if n == 0:
                # c feeds the combines (first needed ~2us after the first
                # expert's matmuls); queue it behind the startup-critical w[0].
                nc.sync.dma_start(
                    c_sb[:], c[:, :].rearrange("(bt p) n -> p bt n", p=P)
                )
                nc.vector.tensor_reduce(
                    t_sb[:], c_sb[:], axis=mybir.AxisListType.X,
                    op=mybir.AluOpType.add,
                )
            if n == 5:
                # x16 feeds only the final relu pass; fetch it once the
                # startup burst is over and the w-stream has caught up.
                nc.sync.dma_start(
                    x16_sb[:], x16[:, :].rearrange("(bt p) i -> p bt i", p=P)
                )
            last = n == N_EXP - 1
            out_r = out[:, :].rearrange("(bt p) o -> p bt o", p=P)
            if n <= 1:
                # DMA-paced fill phase: j-outer over all four batch tiles so
                # the PE consumes each arriving w chunk-pair for every bt at
                # once instead of queueing bt1-3 behind a chunk-stalled bt0.
                zps = [
                    psum.tile([P, NO, FD], F32, name=f"zpj_{bt}", tag="zp")
                    for bt in range(BT)
                ]
                for j in range(NPAIR):
                    for bt in range(BT):
                        for ot in range(NO):
                            nc.tensor.matmul(
                                zps[bt][:, ot],
                                lhsT=xT_sb[:, 2 * j : 2 * j + 2, bt * P : (bt + 1) * P],
                                rhs=w_sb[:, 2 * j : 2 * j + 2, ot * FD : (ot + 1) * FD],
                                start=(j == 0),
                                stop=(j == NPAIR - 1),
                                perf_mode=DR,
                            )
                for bt in range(BT):
                    nc.vector.scalar_tensor_tensor(
                        out=acc[bt][:],
                        in0=zps[bt][:],
                        scalar=c_sb[:, bt, n : n + 1],
                        in1=acc[bt][:],
                        op0=mybir.AluOpType.mult,
                        op1=mybir.AluOpType.add,
                    )
                continue
            for bt in range(BT):
                if not last:
                    zp = psum.tile([P, NO, FD], F32, name="zp", tag="zp")
                    for j in range(NPAIR):
                        for ot in range(NO):
                            nc.tensor.matmul(
                                zp[:, ot],
                                lhsT=xT_sb[:, 2 * j : 2 * j + 2, bt * P : (bt + 1) * P],
                                rhs=w_sb[:, 2 * j : 2 * j + 2, ot * FD : (ot + 1) * FD],
                                start=(j == 0),
                                stop=(j == NPAIR - 1),
                                perf_mode=DR,
                            )
                    # acc += z * c[:, n]  (fused on DVE; c per-partition scalar)
                    nc.vector.scalar_tensor_tensor(
                        out=acc[bt][:],
                        in0=zp[:],
                        scalar=c_sb[:, bt, n : n + 1],
                        in1=acc[bt][:],
                        op0=mybir.AluOpType.mult,
                        op1=mybir.AluOpType.add,
                    )
                    if n == 7:
                        # mid-kernel DVE slack: rowsum(x16) one bt at a time
                        nc.vector.tensor_reduce(
                            s_sb[:, bt : bt + 1],
                            x16_sb[:, bt],
                            axis=mybir.AxisListType.X,
                            op=mybir.AluOpType.add,
                        )
                    if n == 8 and bt == 0:
                        # t = (rowsum(c) * 0.5) * s
                        nc.vector.scalar_tensor_tensor(
                            out=t_sb[:],
                            in0=t_sb[:],
                            scalar=0.5,
                            in1=s_sb[:],
                            op0=mybir.AluOpType.mult,
                            op1=mybir.AluOpType.mult,
                        )
                else:
                    # Last expert: k-major per ot with single-bank PSUM tiles
                    # so each ot's group closes and recycles early; the Scalar
                    # engine applies the mean correction + relu so only the
                    # combine rides the DVE.
                    for ot in range(NO):
                        zp1 = psum.tile([P, FD], F32, name="zp1", tag="zp")
                        for j in range(NPAIR):
                            nc.tensor.matmul(
                                zp1[:],
                                lhsT=xT_sb[:, 2 * j : 2 * j + 2, bt * P : (bt + 1) * P],
                                rhs=w_sb[:, 2 * j : 2 * j + 2, ot * FD : (ot + 1) * FD],
                                start=(j == 0),
                                stop=(j == NPAIR - 1),
                                perf_mode=DR,
                            )
                        nc.vector.scalar_tensor_tensor(
                            out=acc[bt][:, ot],
                            in0=zp1[:],
                            scalar=c_sb[:, bt, n : n + 1],
                            in1=acc[bt][:, ot],
                            op0=mybir.AluOpType.mult,
                            op1=mybir.AluOpType.add,
                        )
                        # out = relu(acc + t)   (Scalar engine, bias per part.)
                        nc.scalar.activation(
                            acc[bt][:, ot],
                            acc[bt][:, ot],
                            mybir.ActivationFunctionType.Relu,
                            bias=t_sb[:, bt : bt + 1],
                            scale=1.0,
                        )
                        nc.sync.dma_start(
                            out_r[:, bt, ot * FD : (ot + 1) * FD],
                            acc[bt][:, ot],
                        )

    nc.compile()
    return nc


_NC_CACHE = {}


def _get_nc():
    if "nc" not in _NC_CACHE:
        _NC_CACHE["nc"] = _build_kernel()
    return _NC_CACHE["nc"]


def _run(x, comp_weight, weight, bias, trace=False):
    x = np.ascontiguousarray(np.asarray(x, dtype=np.float32))
    comp_weight = np.ascontiguousarray(np.asarray(comp_weight, dtype=np.float32))
    weight = np.asarray(weight, dtype=np.float32)
    bias = np.ascontiguousarray(np.asarray(bias, dtype=np.float32))

    # mean-centered fp8 weights: W = 0.5 + V, quantize V in [-0.5, 0.5]
    w_q = np.ascontiguousarray((weight - 0.5).astype(ml_dtypes.float8_e4m3fn))
    in_maps = []
    for r in range(N_CORES):
        sl = slice(r * BS, (r + 1) * BS)
        xs = x[sl]
        cs = comp_weight[sl]
        in_maps.append(
            {
                "xT": np.ascontiguousarray(xs.T).astype(ml_dtypes.float8_e4m3fn),
                "x16": xs.astype(ml_dtypes.bfloat16),
                "w": w_q,
                "c": cs,
                "cT": np.ascontiguousarray(cs.T).astype(ml_dtypes.bfloat16),
                "bias": bias.astype(ml_dtypes.bfloat16),
            }
        )
    res = run_bass_kernel_spmd(
        _get_nc(), in_maps, core_ids=list(range(N_CORES)), trace=trace
    )
    out = np.concatenate([res.results[r]["out"] for r in range(N_CORES)], axis=0)
    return out, res


def kernel(x, comp_weight, weight, bias):
    out, _ = _run(x, comp_weight, weight, bias)
    return out
